# revision 1
# baseline (speedup 1.0000x reference)
"""LDA-loss logits kernel for Trainium2 (8 NeuronCores, SPMD).

Computes logits[b, c] = -0.5 * ||feat[b] - centers[c]||^2
                      = feat[b]·centers[c] - 0.5||feat[b]||^2 - 0.5||centers[c]||^2

Strategy:
  - Shard centers (output columns) across the 8 cores: 10000 classes ->
    1250/core (matmul N-tiles of 512+512+226).
  - Host prep: transpose feat/centers to [D, *] bf16 layouts (contraction on
    partitions), precompute the two squared-norm bias vectors in fp32.
  - Device: everything SBUF-resident. 8x128 K-chunks of bf16 matmuls
    accumulate in fp32 PSUM; eviction adds the per-row bias on ScalarE
    (activation Identity + per-partition bias) and the per-column bias on
    VectorE, then DMAs the fp32 tile out.
"""

import numpy as np
import ml_dtypes

BATCH = 4096
FEAT_DIM = 1024
NUM_CLASSES = 10000
N_CORES = 8
C_PER_REAL = NUM_CLASSES // N_CORES  # 1250
C_PER = 1250                         # padded per-core classes
P = 128
KO = FEAT_DIM // P                   # 8 contraction chunks
MT = BATCH // P                      # 32 output row tiles
N_TILES = ((0, 512), (512, 512), (1024, 226))

_NC = None


def _build_bass():
    import concourse.mybir as mybir
    import concourse.tile as tile
    from concourse import bacc

    nc = bacc.Bacc("TRN2", target_bir_lowering=False, debug=False)

    featT = nc.dram_tensor("featT", [FEAT_DIM, BATCH], mybir.dt.bfloat16,
                           kind="ExternalInput")
    centsT = nc.dram_tensor("centsT", [FEAT_DIM, C_PER], mybir.dt.bfloat16,
                            kind="ExternalInput")
    fsq = nc.dram_tensor("fsq", [P, MT], mybir.dt.float32, kind="ExternalInput")
    csqb = nc.dram_tensor("csqb", [P, C_PER], mybir.dt.float32,
                          kind="ExternalInput")
    out = nc.dram_tensor("out", [BATCH, C_PER], mybir.dt.float32,
                         kind="ExternalOutput")

    with tile.TileContext(nc) as tc:
        _lda_tile_kernel(tc, featT.ap(), centsT.ap(), fsq.ap(), csqb.ap(),
                         out.ap())
    nc.compile()
    return nc


def _lda_tile_kernel(tc, featT, centsT, fsq, csqb, out):
    import concourse.mybir as mybir

    nc = tc.nc
    featT_r = featT.rearrange("(ko p) b -> p ko b", p=P)
    centsT_r = centsT.rearrange("(ko p) c -> p ko c", p=P)
    out_r = out.rearrange("(mo p) c -> p mo c", p=P)

    with (
        tc.tile_pool(name="big", bufs=1) as big,
        tc.tile_pool(name="consts", bufs=1) as consts,
        tc.tile_pool(name="ostage", bufs=16) as ostage,
        tc.tile_pool(name="psum", bufs=6, space="PSUM") as psum,
    ):
        cent_sb = big.tile([P, KO, C_PER], mybir.dt.bfloat16)
        feat_sb = big.tile([P, KO, BATCH], mybir.dt.bfloat16)
        fsq_sb = consts.tile([P, MT], mybir.dt.float32)
        csq_sb = consts.tile([P, C_PER], mybir.dt.float32)

        # All input loads go on the HW-DGE (sync) queue, which sustains full
        # HBM bandwidth when it runs alone. Centers + the first feat m-range
        # load in REVERSE k order: the first matmul needs k=0, which arrives
        # last, so TensorE starts only once its whole first working set is
        # resident and then runs dense — drip-feeding it causes HAM
        # re-throttle stalls that cost more than the later start saves.
        # Output stores use the SW-DGE (gpsimd) queue so they never wait
        # behind the input load in one FIFO.
        MR = 8
        mr_size = BATCH // MR
        for k in range(KO - 1, -1, -1):
            nc.sync.dma_start(cent_sb[:, k], centsT_r[:, k])
            nc.gpsimd.dma_start(feat_sb[:, k, 0:mr_size],
                                featT_r[:, k, 0:mr_size])
        nc.sync.dma_start(fsq_sb[:], fsq)
        nc.sync.dma_start(csq_sb[:], csqb)
        for mr in range(1, MR):
            sl = slice(mr * mr_size, (mr + 1) * mr_size)
            for k in range(KO):
                nc.sync.dma_start(feat_sb[:, k, sl], featT_r[:, k, sl])

        for m in range(MT):
            msl = slice(m * P, (m + 1) * P)
            for n0, nsz in N_TILES:
                ps = psum.tile([P, 512], mybir.dt.float32, tag="ps",
                               name="ps")[:, :nsz]
                for k in range(KO):
                    nc.tensor.matmul(
                        ps,
                        feat_sb[:, k, msl],
                        cent_sb[:, k, n0:n0 + nsz],
                        start=(k == 0),
                        stop=(k == KO - 1),
                    )
                ot = ostage.tile([P, 512], mybir.dt.float32, tag="ot",
                                 name="ot")[:, :nsz]
                # ot = psum + fsq[row]  (per-partition bias on ScalarE)
                nc.scalar.activation(
                    ot, ps, mybir.ActivationFunctionType.Identity,
                    bias=fsq_sb[:, m:m + 1],
                )
                # ot += csq[col]  (per-column bias on VectorE)
                nc.vector.tensor_add(ot, ot, csq_sb[:, n0:n0 + nsz])
                eng = nc.gpsimd if (m + n0) % 2 else nc.sync
                eng.dma_start(out_r[:, m, n0:n0 + nsz], ot)


def _get_nc():
    global _NC
    if _NC is None:
        _NC = _build_bass()
    return _NC


def _prep_inputs(feat, centers):
    feat = np.asarray(feat, dtype=np.float32)
    centers = np.asarray(centers, dtype=np.float32)

    featT_bf = np.ascontiguousarray(feat.T).astype(ml_dtypes.bfloat16)
    fsq_v = -0.5 * np.einsum("bd,bd->b", feat, feat)
    fsq_mat = np.ascontiguousarray(fsq_v.reshape(MT, P).T)  # [P, MT]

    in_maps = []
    for i in range(N_CORES):
        cs = centers[i * C_PER_REAL:(i + 1) * C_PER_REAL]
        centsT_bf = np.zeros((FEAT_DIM, C_PER), dtype=ml_dtypes.bfloat16)
        centsT_bf[:, :C_PER_REAL] = cs.T.astype(ml_dtypes.bfloat16)
        csq = np.zeros(C_PER, dtype=np.float32)
        csq[:C_PER_REAL] = -0.5 * np.einsum("cd,cd->c", cs, cs)
        csqb = np.ascontiguousarray(
            np.broadcast_to(csq[None, :], (P, C_PER)))
        in_maps.append({
            "featT": featT_bf,
            "centsT": centsT_bf,
            "fsq": fsq_mat,
            "csqb": csqb,
        })
    return in_maps


def _run(inputs, trace=False, trace_cores=None):
    from concourse import bass_utils

    nc = _get_nc()
    in_maps = _prep_inputs(inputs["feat"], inputs["centers"])
    res = bass_utils.run_bass_kernel_spmd(
        nc, in_maps, core_ids=list(range(N_CORES)), trace=trace,
        trace_cores=trace_cores,
    )
    shards = [res.results[i]["out"][:, :C_PER_REAL] for i in range(N_CORES)]
    full = np.concatenate(shards, axis=1)
    return full, res


def kernel(**inputs) -> np.ndarray:
    return _run(inputs)[0]



# revision 2
# speedup vs baseline: 1.0138x; 1.0138x over previous
"""LDA-loss logits kernel for Trainium2 (8 NeuronCores, SPMD) — fp8 DoubleRow v3.

v3 over v2:
  - Norm biases (-0.5||f||^2, -0.5||c||^2) added on HOST during unshard;
    device emits pure fp8 cross-term GEMM -> fp16. (Finer fp16 grid too:
    |cross| ~ 180 vs |logits| ~ 1100.)
  - C_PER 1280 -> 1264 (=79*16): 1.2% less padded matmul work.
  - Head: first matmul gated only on centers + feat range 0 (~1.75MB);
    feat ranges 1..7 are queued on the gpsimd queue BEHIND the m=1,3,..,13
    output stores, so their transfers can't steal HBM bandwidth from the
    critical first working set.
  - Tail: last two m-rows stored per n-tile (smaller final transfers).
"""

import numpy as np
import ml_dtypes

BATCH = 4096
FEAT_DIM = 1024
NUM_CLASSES = 10000
N_CORES = 8
C_PER_REAL = NUM_CLASSES // N_CORES  # 1250
C_PER = 1264                         # padded per-core classes (79*16)
P = 128
KP = 4                               # contraction k-pairs of 256
MT = BATCH // P                      # 32 output row tiles
MR = 8                               # feat DMA ranges
MR_SZ = BATCH // MR                  # 512 rows per range
N_TILES = ((0, 512), (512, 512), (1024, 240))

_NC = None


def _build_bass():
    import concourse.mybir as mybir
    import concourse.tile as tile
    from concourse import bacc

    nc = bacc.Bacc("TRN2", target_bir_lowering=False, debug=False)

    featP = nc.dram_tensor("featP", [P, MR, KP, 2, MR_SZ], mybir.dt.float8e4,
                           kind="ExternalInput")
    centsP = nc.dram_tensor("centsP", [P, KP, 2, C_PER], mybir.dt.float8e4,
                            kind="ExternalInput")
    out = nc.dram_tensor("out", [BATCH, C_PER], mybir.dt.float16,
                         kind="ExternalOutput")

    with tile.TileContext(nc) as tc:
        _lda_tile_kernel(tc, featP.ap(), centsP.ap(), out.ap())
    nc.compile()
    return nc


def _lda_tile_kernel(tc, featP, centsP, out):
    import concourse.mybir as mybir

    nc = tc.nc
    out_r = out.rearrange("(mo p) c -> p mo c", p=P)

    with (
        tc.tile_pool(name="big", bufs=1) as big,
        tc.tile_pool(name="ostage", bufs=4) as ostage,
        tc.tile_pool(name="psum", bufs=6, space="PSUM") as psum,
    ):
        cent_sb = big.tile([P, KP, 2, C_PER], mybir.dt.float8e4)
        feat_sb = big.tile([P, MR, KP, 2, MR_SZ], mybir.dt.float8e4)

        # Gate working set: centers (sync queue) + feat range 0 (gpsimd).
        nc.sync.dma_start(cent_sb[:], centsP)
        nc.gpsimd.dma_start(feat_sb[:, 0], featP[:, 0])
        pending_load = 1

        for m in range(MT):
            r, mq = divmod(m, MT // MR)
            msl = slice(mq * P, (mq + 1) * P)
            ot = ostage.tile([P, C_PER], mybir.dt.float16, tag="ot", name="ot")
            split_store = m >= MT - 2
            for ni, (n0, nsz) in enumerate(N_TILES):
                ps = psum.tile([P, 512], mybir.dt.float32, tag="ps",
                               name="ps")[:, :nsz]
                for j in range(KP):
                    nc.tensor.matmul(
                        ps,
                        feat_sb[:, r, j, :, msl],
                        cent_sb[:, j, :, n0:n0 + nsz],
                        start=(j == 0),
                        stop=(j == KP - 1),
                        perf_mode=mybir.MatmulPerfMode.DoubleRow,
                    )
                # pure dtype cast fp32 -> fp16; biases are added on host
                osl = ot[:, n0:n0 + nsz]
                if ni == 1:
                    nc.vector.tensor_copy(osl, ps)
                else:
                    nc.scalar.activation(
                        osl, ps, mybir.ActivationFunctionType.Identity)
                if split_store:
                    eng = nc.gpsimd if m % 2 else nc.sync
                    eng.dma_start(out_r[:, m, n0:n0 + nsz], osl)
            if not split_store:
                eng = nc.gpsimd if m % 2 else nc.sync
                eng.dma_start(out_r[:, m], ot)
            # Queue the next feat range on the gpsimd queue behind this
            # store: its transfer starts only once compute has progressed,
            # keeping early HBM bandwidth for the critical path.
            if m % 2 and pending_load < MR:
                nc.gpsimd.dma_start(feat_sb[:, pending_load],
                                    featP[:, pending_load])
                pending_load += 1


def _get_nc():
    global _NC
    if _NC is None:
        _NC = _build_bass()
    return _NC


def _prep_inputs(feat, centers):
    feat = np.asarray(feat, dtype=np.float32)
    centers = np.asarray(centers, dtype=np.float32)

    f8 = feat.astype(ml_dtypes.float8_e4m3)
    # featP[p, r, j, t, b] = fp8(feat[r*512 + b, j*256 + t*128 + p])
    featP = np.ascontiguousarray(
        f8.T.reshape(KP, 2, P, MR, MR_SZ).transpose(2, 3, 0, 1, 4))

    in_maps = []
    for i in range(N_CORES):
        cs = centers[i * C_PER_REAL:(i + 1) * C_PER_REAL]
        c8 = cs.astype(ml_dtypes.float8_e4m3)
        centsP = np.zeros((P, KP, 2, C_PER), dtype=ml_dtypes.float8_e4m3)
        centsP[:, :, :, :C_PER_REAL] = (
            c8.T.reshape(KP, 2, P, C_PER_REAL).transpose(2, 0, 1, 3))
        in_maps.append({"featP": featP, "centsP": centsP})
    return in_maps


def _run(inputs, trace=False, trace_cores=None):
    from concourse import bass_utils

    feat = np.asarray(inputs["feat"], dtype=np.float32)
    centers = np.asarray(inputs["centers"], dtype=np.float32)

    nc = _get_nc()
    in_maps = _prep_inputs(feat, centers)
    res = bass_utils.run_bass_kernel_spmd(
        nc, in_maps, core_ids=list(range(N_CORES)), trace=trace,
        trace_cores=trace_cores,
    )
    shards = [res.results[i]["out"][:, :C_PER_REAL] for i in range(N_CORES)]
    full = np.concatenate(shards, axis=1).astype(np.float32)
    # logits = cross - 0.5||f||^2 - 0.5||c||^2 (norm biases exact in fp32)
    full += (-0.5 * np.einsum("bd,bd->b", feat, feat))[:, None]
    full += (-0.5 * np.einsum("cd,cd->c", centers, centers))[None, :]
    return full, res


def kernel(**inputs) -> np.ndarray:
    return _run(inputs)[0]


# revision 4
# speedup vs baseline: 1.0381x; 1.0240x over previous
"""LDA-loss logits kernel for Trainium2 (8 NeuronCores, SPMD) — fp8 DoubleRow v9.

v9 over v5: the head loop is re-phased to match DMA arrival order, so real
compute starts on the first 0.625MB (centers cols 0:512 + feat rows 0:256)
instead of idling until the full 2.25MB working set lands:
  phase 1: (m0,n0), (m1,n0)            <- needs cents[0:512],  feat[0:256]
  phase 2: (m0,n1), (m1,n1)            <- needs cents[512:1024]
           (m0,n2), (m1,n2)            <- needs cents[1024:1264]
  dense:   m2..m31 all n               <- feat[256:512] arrives meanwhile
The warm-up count drops to 12 (just bridging to the smaller gate); there is
no idle gap afterwards, so the PE p-state keeps ramping on real work.
Warm-up PSUM reuses the main pool tag (one fewer pool to tear down).
"""

import numpy as np
import ml_dtypes

BATCH = 4096
FEAT_DIM = 1024
NUM_CLASSES = 10000
N_CORES = 8
C_PER_REAL = NUM_CLASSES // N_CORES  # 1250
C_PER = 1264                         # padded per-core classes (79*16)
P = 128
KP = 4                               # contraction k-pairs of 256
MT = BATCH // P                      # 32 output row tiles
MR = 8                               # feat DMA ranges
MR_SZ = BATCH // MR                  # 512 rows per range
N_TILES = ((0, 512), (512, 512), (1024, 240))
N_WARMUP = 12

_NC = None


def _build_bass():
    import concourse.mybir as mybir
    import concourse.tile as tile
    from concourse import bacc

    nc = bacc.Bacc("TRN2", target_bir_lowering=False, debug=False)

    featP = nc.dram_tensor("featP", [P, MR, KP, 2, MR_SZ], mybir.dt.float8e4,
                           kind="ExternalInput")
    centsP = nc.dram_tensor("centsP", [P, KP, 2, C_PER], mybir.dt.float8e4,
                            kind="ExternalInput")
    out = nc.dram_tensor("out", [BATCH, C_PER], mybir.dt.float16,
                         kind="ExternalOutput")

    with tile.TileContext(nc) as tc:
        _lda_tile_kernel(tc, featP.ap(), centsP.ap(), out.ap())
    nc.compile()
    return nc


def _lda_tile_kernel(tc, featP, centsP, out):
    import concourse.mybir as mybir

    nc = tc.nc
    out_r = out.rearrange("(mo p) c -> p mo c", p=P)

    with (
        tc.tile_pool(name="big", bufs=1) as big,
        tc.tile_pool(name="ostage", bufs=4) as ostage,
        tc.tile_pool(name="psum", bufs=6, space="PSUM") as psum,
    ):
        cent_sb = big.tile([P, KP, 2, C_PER], mybir.dt.float8e4)
        feat_sb = big.tile([P, MR, KP, 2, MR_SZ], mybir.dt.float8e4)
        warm_in = big.tile([P, 2, 256], mybir.dt.float8e4)

        # PE warm-up: dummy matmuls with no input deps run during the head
        # DMA wait, so real matmuls start at speed. Same pool tag as the
        # real accumulators — it just takes one rotation slot.
        nc.vector.memset(warm_in[:], 0)
        warm_ps = psum.tile([P, 512], mybir.dt.float32, tag="ps", name="ps")
        for _ in range(N_WARMUP):
            nc.tensor.matmul(warm_ps[:, :256], warm_in[:, :, :128], warm_in,
                             start=True, stop=True,
                             perf_mode=mybir.MatmulPerfMode.DoubleRow)

        # Loads ordered by first use (all on the sync HW-DGE queue).
        nc.sync.dma_start(cent_sb[:, :, :, 0:512], centsP[:, :, :, 0:512])
        nc.sync.dma_start(feat_sb[:, 0, :, :, 0:256], featP[:, 0, :, :, 0:256])
        nc.sync.dma_start(cent_sb[:, :, :, 512:1024],
                          centsP[:, :, :, 512:1024])
        nc.sync.dma_start(cent_sb[:, :, :, 1024:C_PER],
                          centsP[:, :, :, 1024:])
        nc.sync.dma_start(feat_sb[:, 0, :, :, 256:], featP[:, 0, :, :, 256:])
        pending_load = [1]

        schedule = [(0, 0), (1, 0), (0, 1), (1, 1), (0, 2), (1, 2)]
        for m in range(2, MT):
            schedule += [(m, 0), (m, 1), (m, 2)]

        ots = {}
        ndone = {}

        def finish_row(m, ot):
            nc.sync.dma_start(out_r[:, m], ot)
            # FIFO gating: this load sits behind the store above in the
            # sync queue, so its HBM traffic starts only once compute has
            # reached m — early bandwidth stays on the critical path.
            if m % 2 == 0 and pending_load[0] < MR:
                nc.sync.dma_start(feat_sb[:, pending_load[0]],
                                  featP[:, pending_load[0]])
                pending_load[0] += 1

        for m, ni in schedule:
            n0, nsz = N_TILES[ni]
            r, mq = divmod(m, MT // MR)
            msl = slice(mq * P, (mq + 1) * P)
            if m not in ots:
                ots[m] = ostage.tile([P, C_PER], mybir.dt.float16, tag="ot",
                                     name="ot")
                ndone[m] = 0
            ot = ots[m]
            ps = psum.tile([P, 512], mybir.dt.float32, tag="ps",
                           name="ps")[:, :nsz]
            for j in range(KP):
                nc.tensor.matmul(
                    ps,
                    feat_sb[:, r, j, :, msl],
                    cent_sb[:, j, :, n0:n0 + nsz],
                    start=(j == 0),
                    stop=(j == KP - 1),
                    perf_mode=mybir.MatmulPerfMode.DoubleRow,
                )
            # pure dtype cast fp32 -> fp16; biases are added on host
            osl = ot[:, n0:n0 + nsz]
            if ni == 1:
                nc.vector.tensor_copy(osl, ps)
            else:
                nc.scalar.activation(
                    osl, ps, mybir.ActivationFunctionType.Identity)
            ndone[m] += 1
            if ndone[m] == len(N_TILES):
                finish_row(m, ot)
                del ots[m]


def _get_nc():
    global _NC
    if _NC is None:
        _NC = _build_bass()
    return _NC


def _prep_inputs(feat, centers):
    feat = np.asarray(feat, dtype=np.float32)
    centers = np.asarray(centers, dtype=np.float32)

    f8 = feat.astype(ml_dtypes.float8_e4m3)
    # featP[p, r, j, t, b] = fp8(feat[r*512 + b, j*256 + t*128 + p])
    featP = np.ascontiguousarray(
        f8.T.reshape(KP, 2, P, MR, MR_SZ).transpose(2, 3, 0, 1, 4))

    in_maps = []
    for i in range(N_CORES):
        cs = centers[i * C_PER_REAL:(i + 1) * C_PER_REAL]
        c8 = cs.astype(ml_dtypes.float8_e4m3)
        centsP = np.zeros((P, KP, 2, C_PER), dtype=ml_dtypes.float8_e4m3)
        centsP[:, :, :, :C_PER_REAL] = (
            c8.T.reshape(KP, 2, P, C_PER_REAL).transpose(2, 0, 1, 3))
        in_maps.append({"featP": featP, "centsP": centsP})
    return in_maps


def _run(inputs, trace=False, trace_cores=None):
    from concourse import bass_utils

    feat = np.asarray(inputs["feat"], dtype=np.float32)
    centers = np.asarray(inputs["centers"], dtype=np.float32)

    nc = _get_nc()
    in_maps = _prep_inputs(feat, centers)
    res = bass_utils.run_bass_kernel_spmd(
        nc, in_maps, core_ids=list(range(N_CORES)), trace=trace,
        trace_cores=trace_cores,
    )
    shards = [res.results[i]["out"][:, :C_PER_REAL] for i in range(N_CORES)]
    full = np.concatenate(shards, axis=1).astype(np.float32)
    # logits = cross - 0.5||f||^2 - 0.5||c||^2 (norm biases exact in fp32)
    full += (-0.5 * np.einsum("bd,bd->b", feat, feat))[:, None]
    full += (-0.5 * np.einsum("cd,cd->c", centers, centers))[None, :]
    return full, res


def kernel(**inputs) -> np.ndarray:
    return _run(inputs)[0]
